# revision 2
# baseline (speedup 1.0000x reference)
"""Two-layer LSTM (B=256, T=256, D=128, H=1024, O=128) on 8 trn2 NeuronCores.

v4 = v3 (topology-aware 2x4 sharding: batch halves across core groups
[[0-3],[4-7]], 4-way H split within a group, per-layer in-group AllGathers
hidden under the other layer's matmuls) plus critical-path work on the cell:

- The stored state is H' = 2h and C' = 2c, so every sigmoid becomes
  0.5*(1+tanh(x/2)) with the (1+t) folded into fused scalar_tensor_tensor
  vector ops and the 0.5 folded EXACTLY into the fp16 weights that consume h
  (W_hh0, W_ih1, W_hh1, W_out).  All activations are Tanh (no ACT table
  churn), same vector-op count as the plain cell:
      u = (tf + 1) * C';  v = (ti + 1) * tg
      C'new = 0.5*u + v;  tc = tanh(0.5*C'new);  H'new = (to + 1) * tc
- The whole post-gate chain runs at [128,128] half-tile granularity and the
  collective staging DMA is split per half, so the gather launches ~1us
  earlier (the chain of cross-engine hops costs ~0.5-1us each).

PSUM: one start=True per 2KB bank per step (start clears has_written for
the whole bank); gate regions pack 4-per-bank.
"""

import numpy as np

import concourse.bass as bass
import concourse.mybir as mybir
import concourse.tile as tile
from concourse.bass_utils import run_bass_kernel_spmd

B, T, D, H, O = 256, 256, 128, 1024, 128
NC = 8
GW = 4                # group width (cores per batch-half group)
HC = H // GW          # 256 h rows per core
B2 = B // 2           # 128 batch cols per group
KH = H // 128         # 8 k-chunks over H
NR = 8                # M-tiles per layer per core: (gate, sub-tile t)
FP = mybir.dt.float16
F32 = mybir.dt.float32
AFT = mybir.ActivationFunctionType
ALU = mybir.AluOpType
RG = [[0, 1, 2, 3], [4, 5, 6, 7]]


def _act_block(nc, apool, gp, bb, c_sb, tag):
    """All-tanh LSTM cell tail for my 256 h rows x 128 batch, processed per
    128-col half-tile t so the chain pipelines.

    gp: 2 psum tiles [128, 512]; region r = gamma*2+t (tile r//4, col region
    r%4) holds gate gamma half t pre-activations WITHOUT bias.  bb: [128,NR]
    fp32; cols for i/f/o gates hold bias/2, g-gate cols hold full bias.
    c_sb: C' = 2c [128, 256] fp32 (col t*128+b), updated in place.
    Returns H' = 2h [128, 256] fp16."""
    t_ = [apool.tile([128, 256], F32, tag=tag + n, name=tag + n)
          for n in "ifgo"]
    it, ft, gt, ot = t_
    u = apool.tile([128, 256], F32, tag=tag + "u")
    v = apool.tile([128, 256], F32, tag=tag + "v")
    tc_ = apool.tile([128, 256], F32, tag=tag + "tc")
    hnew = apool.tile([128, 256], FP, tag=tag + "h")
    sls = [slice(0, 128), slice(128, 256)]

    def reg(gamma, t):
        r = gamma * 2 + t
        return gp[r // 4][:, (r % 4) * 128:(r % 4 + 1) * 128], bb[:, r:r + 1]

    # tf = tanh((gf+bf)/2); ti, to likewise; tg = tanh(gg+bg).  All gate
    # acts first (their psum regions complete in this order), then the
    # fused vector chain per half, then tanh(C') per half, then H' per
    # half — so the two halves pipeline across scalar/vector queues.
    for gamma, dst in ((0, it), (1, ft), (2, gt), (3, ot)):
        for t in (0, 1):
            pre, bias = reg(gamma, t)
            nc.scalar.activation(dst[:, sls[t]], pre, AFT.Tanh, bias=bias,
                                 scale=1.0 if gamma == 2 else 0.5)
    for t in (0, 1):
        sl = sls[t]
        nc.vector.scalar_tensor_tensor(
            u[:, sl], ft[:, sl], 1.0, c_sb[:, sl], ALU.add, ALU.mult)
        nc.vector.scalar_tensor_tensor(
            v[:, sl], it[:, sl], 1.0, gt[:, sl], ALU.add, ALU.mult)
        nc.vector.scalar_tensor_tensor(
            c_sb[:, sl], u[:, sl], 0.5, v[:, sl], ALU.mult, ALU.add)
    for t in (0, 1):
        nc.scalar.activation(tc_[:, sls[t]], c_sb[:, sls[t]], AFT.Tanh,
                             scale=0.5)
    for t in (0, 1):
        nc.vector.scalar_tensor_tensor(
            hnew[:, sls[t]], ot[:, sls[t]], 1.0, tc_[:, sls[t]],
            ALU.add, ALU.mult)
    return hnew


def build(t_steps, debug=False, no_cc=False):
    nc = bass.Bass(num_devices=NC)

    # all tensors are per-core pre-sliced SBUF images (contiguous DMAs)
    xg = nc.dram_tensor("xg", [t_steps, D, B2], FP, kind="ExternalInput")
    w0i = nc.dram_tensor("w0i", [NR, 128, 128], FP, kind="ExternalInput")
    w0h = nc.dram_tensor("w0h", [KH * NR, 128, 128], FP, kind="ExternalInput")
    w1i = nc.dram_tensor("w1i", [KH * NR, 128, 128], FP, kind="ExternalInput")
    w1h = nc.dram_tensor("w1h", [KH * NR, 128, 128], FP, kind="ExternalInput")
    wo = nc.dram_tensor("wo", [KH, 128, O], FP, kind="ExternalInput")
    b0 = nc.dram_tensor("b0", [128, NR], F32, kind="ExternalInput")
    b1 = nc.dram_tensor("b1", [128, NR], F32, kind="ExternalInput")
    bo = nc.dram_tensor("bo", [1, O], FP, kind="ExternalInput")
    ones = nc.dram_tensor("ones", [1, B2], FP, kind="ExternalInput")
    h0i = nc.dram_tensor("h0i", [128, KH * 128], FP, kind="ExternalInput")
    c0i = nc.dram_tensor("c0i", [128, 256], F32, kind="ExternalInput")
    outT = nc.dram_tensor("outT", [O, B2], F32, kind="ExternalOutput")
    if debug:
        dbg_h0 = nc.dram_tensor("dbg_h0", [128, KH * 128], FP, kind="ExternalOutput")
        dbg_h1 = nc.dram_tensor("dbg_h1", [128, KH * 128], FP, kind="ExternalOutput")

    with tile.TileContext(nc) as tc:
        with (
            tc.tile_pool(name="wpool", bufs=1) as wpool,
            tc.tile_pool(name="spool", bufs=1) as spool,
            tc.tile_pool(name="xpool", bufs=6) as xpool,
            tc.tile_pool(name="apool", bufs=3) as apool,
            tc.tile_pool(name="ppool", bufs=1, space="PSUM") as ppool,
            tc.tile_pool(name="dpool", bufs=4, space="DRAM") as dpool,
        ):
            w0i_sb = wpool.tile([128, NR * 128], FP, tag="w0i")
            w0h_sb = wpool.tile([128, KH * NR * 128], FP, tag="w0h")
            w1i_sb = wpool.tile([128, KH * NR * 128], FP, tag="w1i")
            w1h_sb = wpool.tile([128, KH * NR * 128], FP, tag="w1h")
            wo_sb = wpool.tile([128, KH * O], FP, tag="wo")
            b0_sb = wpool.tile([128, NR], F32, tag="b0")
            b1_sb = wpool.tile([128, NR], F32, tag="b1")
            bo_sb = wpool.tile([1, O], FP, tag="bo")
            ones_sb = wpool.tile([1, B2], FP, tag="ones")
            nc.sync.dma_start(w0i_sb[:].rearrange("p (r m) -> p r m", r=NR), w0i[:].rearrange("r p m -> p r m"))
            nc.sync.dma_start(w0h_sb[:].rearrange("p (q m) -> p q m", q=KH * NR), w0h[:].rearrange("q p m -> p q m"))
            nc.sync.dma_start(w1i_sb[:].rearrange("p (q m) -> p q m", q=KH * NR), w1i[:].rearrange("q p m -> p q m"))
            nc.sync.dma_start(w1h_sb[:].rearrange("p (q m) -> p q m", q=KH * NR), w1h[:].rearrange("q p m -> p q m"))
            nc.sync.dma_start(wo_sb[:].rearrange("p (k m) -> p k m", k=KH), wo[:].rearrange("k p m -> p k m"))
            nc.sync.dma_start(b0_sb[:], b0[:])
            nc.sync.dma_start(b1_sb[:], b1[:])
            nc.sync.dma_start(bo_sb[:], bo[:])
            nc.sync.dma_start(ones_sb[:], ones[:])

            # h state double buffers [128, KH*128] (col (w*2+t)*128+b):
            # tick s reads buf s%2, gathers of tick s land in buf (s+1)%2
            h0_sb = [spool.tile([128, KH * 128], FP, tag=f"h0T{i}",
                                name=f"h0T{i}") for i in (0, 1)]
            h1_sb = [spool.tile([128, KH * 128], FP, tag=f"h1T{i}",
                                name=f"h1T{i}") for i in (0, 1)]
            c0_sb = spool.tile([128, 256], F32, tag="c0")
            c1_sb = spool.tile([128, 256], F32, tag="c1")
            nc.sync.dma_start(h0_sb[0][:], h0i[:])
            nc.sync.dma_start(h1_sb[1][:], h0i[:])
            nc.sync.dma_start(c0_sb[:], c0i[:])
            nc.sync.dma_start(c1_sb[:], c0i[:])

            def gates(gp, wi_sb, wh_sb, hin, hrec, n_in_k):
                """Accumulate gate pre-activations for my NR M-tiles.
                hin-dependent matmuls first, hrec-dependent last.

                start=True clears has_written bits for the ENTIRE 2KB PSUM
                bank, so issue exactly one start per bank (first matmul into
                it) and one stop (its last matmul)."""
                for r in range(NR):
                    for k in range(n_in_k):
                        nc.tensor.matmul(
                            gp[r // 4][:, (r % 4) * 128:(r % 4 + 1) * 128],
                            wi_sb[:, (k * NR + r) * 128:(k * NR + r + 1) * 128],
                            hin[:, k * 128:(k + 1) * 128],
                            start=(k == 0 and r % 4 == 0), stop=False)
                for r in range(NR):
                    for k in range(KH):
                        nc.tensor.matmul(
                            gp[r // 4][:, (r % 4) * 128:(r % 4 + 1) * 128],
                            wh_sb[:, (k * NR + r) * 128:(k * NR + r + 1) * 128],
                            hrec[:, k * 128:(k + 1) * 128],
                            start=False,
                            stop=(k == KH - 1 and r % 4 == 3))

            def gather(hnew, h_dst, cci_tag, cco_tag):
                cc_in = dpool.tile([128, 256], FP, tag=cci_tag, name=cci_tag)
                cc_out = dpool.tile([GW, 128, 256], FP, tag=cco_tag, name=cco_tag)
                # stage per half-tile so the DMA starts as soon as that half
                # of the chain finishes
                nc.gpsimd.dma_start(cc_in[:, :128], hnew[:, :128])
                nc.gpsimd.dma_start(cc_in[:, 128:], hnew[:, 128:])
                if not no_cc:
                    nc.gpsimd.collective_compute(
                        "AllGather", mybir.AluOpType.bypass, replica_groups=RG,
                        ins=[cc_in.opt()], outs=[cc_out.opt()])
                nc.sync.dma_start(
                    h_dst[:].rearrange("p (w x) -> p w x", w=GW),
                    cc_out[:].rearrange("w p x -> p w x"))

            for s in range(t_steps + 1):
                p = s % 2
                q = 1 - p
                if s < t_steps:
                    xt = xpool.tile([D, B2], FP, tag="xt")
                    nc.scalar.dma_start(xt[:], xg[s])
                    gp0 = [ppool.tile([128, 512], F32, tag=f"p0{i}",
                                      name=f"p0{i}_{s}") for i in (0, 1)]
                    gates(gp0, w0i_sb, w0h_sb, xt, h0_sb[p], 1)
                    h0new = _act_block(nc, apool, gp0, b0_sb, c0_sb, "l0")
                    gather(h0new, h0_sb[q], "cc0i", "cc0o")
                if s >= 1:
                    gp1 = [ppool.tile([128, 512], F32, tag=f"p1{i}",
                                      name=f"p1{i}_{s}") for i in (0, 1)]
                    gates(gp1, w1i_sb, w1h_sb, h0_sb[p], h1_sb[p], KH)
                    h1new = _act_block(nc, apool, gp1, b1_sb, c1_sb, "l1")
                    gather(h1new, h1_sb[q], "cc1i", "cc1o")

            # output projection for my half: out^T[O, B2] = W_out' @ H1' + b_out
            pfin = (t_steps + 1) % 2
            po = ppool.tile([O, B2], F32, tag="po")
            nc.tensor.matmul(po[:], bo_sb[:], ones_sb[:], start=True, stop=False)
            for k in range(KH):
                nc.tensor.matmul(
                    po[:], wo_sb[:, k * O:(k + 1) * O],
                    h1_sb[pfin][:, k * 128:(k + 1) * 128],
                    start=False, stop=(k == KH - 1))
            out_sb = apool.tile([O, B2], F32, tag="out")
            nc.scalar.copy(out_sb[:], po[:])
            nc.sync.dma_start(outT[:], out_sb[:])
            if debug:
                nc.sync.dma_start(dbg_h0[:], h0_sb[t_steps % 2][:])
                nc.sync.dma_start(dbg_h1[:], h1_sb[pfin][:])

    _split_excess_waits(nc)
    return nc


def _split_excess_waits(nc):
    """This walrus build embeds at most ONE sync wait per instruction (any
    type).  Move excess waits onto same-engine drains inserted immediately
    before the instruction, one wait per drain — engine queues execute in
    order, so semantics are unchanged."""
    for bb in nc.main_func.blocks:
        insts = list(bb.instructions)
        inserts = {}
        extras = []
        for pos, ins in enumerate(insts):
            si = ins.sync_info
            if si is None or not si.on_wait or len(si.on_wait) <= 1:
                continue
            waits = list(si.on_wait)
            keep, excess = waits[-1:], waits[:-1]
            carriers = []
            for w in excess:
                d = nc.engines[ins.engine].drain(fusable=False).ins
                d.sync_info = mybir.SyncInfo(on_wait=[w], on_update=[])
                carriers.append(d)
                extras.append(d)
            inserts[pos] = carriers
            si.on_wait = keep
            ins.sync_info = si
        if not inserts:
            continue
        extra_set = set(id(e) for e in extras)
        for blk in nc.main_func.blocks:
            blk.instructions = [i for i in blk.instructions
                                if id(i) not in extra_set]
        out = []
        for pos, ins in enumerate(insts):
            out.extend(inserts.get(pos, ()))
            out.append(ins)
        bb.instructions = out


def make_in_maps(x, h0, c0, W_ih0, W_hh0, b_ih0, b_hh0,
                 W_ih1, W_hh1, b_ih1, b_hh1, W_out, b_out, t_steps):
    """Per-core SBUF images.  Core j: group g=j//4 (batch half), member
    m=j%4 (owns h rows [m*256, (m+1)*256)).  State is scaled: H'=2h, C'=2c;
    weights that consume h carry a 0.5 factor (exact in fp16)."""
    wo_host = np.ascontiguousarray(
        0.5 * W_out.T).astype(np.float16).reshape(KH, 128, O)
    bo_host = b_out.astype(np.float16).reshape(1, O)

    def wimg(W, m, kin, scale):
        out = np.empty((kin * NR, 128, 128), np.float16)
        for k in range(kin):
            for r in range(NR):
                gamma, t = r // 2, r % 2
                rows = slice(gamma * H + m * HC + t * 128,
                             gamma * H + m * HC + (t + 1) * 128)
                out[k * NR + r] = (
                    scale * W[rows, k * 128:(k + 1) * 128].T).astype(np.float16)
        return out

    def bimg(bvec, m):
        # [128, NR] fp32; i/f/o-gate cols pre-halved (sigmoid-via-tanh),
        # g-gate cols full
        out = np.empty((128, NR), np.float32)
        for r in range(NR):
            gamma, t = r // 2, r % 2
            bb = bvec[gamma * H + m * HC + t * 128:
                      gamma * H + m * HC + (t + 1) * 128]
            out[:, r] = bb if gamma == 2 else 0.5 * bb
        return out

    def himg(hT_half):
        # H' = 2*h initial, [128, KH*128] col k*128+b <- 2*hT[k*128+p, b]
        return np.ascontiguousarray(
            (2.0 * hT_half).reshape(KH, 128, B2).transpose(1, 0, 2)
            .reshape(128, KH * B2)).astype(np.float16)

    def cimg(cT_half, m):
        # C' = 2*c initial, my 256 rows, [128, 256] col t*128+b
        chunk = 2.0 * cT_half[m * HC:(m + 1) * HC]
        return np.ascontiguousarray(
            chunk.reshape(2, 128, B2).transpose(1, 0, 2).reshape(128, 256)
        ).astype(np.float32)

    h0T = h0.T.astype(np.float32)
    c0T = c0.T.astype(np.float32)
    b0v = (b_ih0 + b_hh0).astype(np.float32)
    b1v = (b_ih1 + b_hh1).astype(np.float32)
    xT = np.transpose(x[:, :t_steps, :], (1, 2, 0)).astype(np.float16)

    in_maps = []
    for j in range(NC):
        g, m = j // GW, j % GW
        sl = slice(g * B2, (g + 1) * B2)
        in_maps.append({
            "xg": np.ascontiguousarray(xT[:, :, sl]),
            "w0i": wimg(W_ih0, m, 1, 1.0),
            "w0h": wimg(W_hh0, m, KH, 0.5),
            "w1i": wimg(W_ih1, m, KH, 0.5),
            "w1h": wimg(W_hh1, m, KH, 0.5),
            "wo": wo_host, "bo": bo_host,
            "b0": bimg(b0v, m), "b1": bimg(b1v, m),
            "ones": np.ones((1, B2), np.float16),
            "h0i": himg(h0T[:, sl]),
            "c0i": cimg(c0T[:, sl], m),
        })
    return in_maps


def run(t_steps, in_maps, trace=False):
    nc = build(t_steps)
    res = run_bass_kernel_spmd(nc, in_maps, list(range(NC)), trace=trace)
    return res


def assemble(res):
    out = np.concatenate(
        [res.results[0]["outT"].T, res.results[GW]["outT"].T], axis=0)
    return np.ascontiguousarray(out).astype(np.float32)


def kernel(**inputs):
    args = {k: np.asarray(v) for k, v in inputs.items()}
    in_maps = make_in_maps(
        args["x"], args["h0"], args["c0"],
        args["W_ih0"], args["W_hh0"], args["b_ih0"], args["b_hh0"],
        args["W_ih1"], args["W_hh1"], args["b_ih1"], args["b_hh1"],
        args["W_out"], args["b_out"], T)
    res = run(T, in_maps)
    return assemble(res)


# revision 3
# speedup vs baseline: 1.2499x; 1.2499x over previous
"""Two-layer LSTM (B=256, T=256, D=128, H=1024, O=128) on 8 trn2 NeuronCores.

v4 = v3 (topology-aware 2x4 sharding: batch halves across core groups
[[0-3],[4-7]], 4-way H split within a group, per-layer in-group AllGathers
hidden under the other layer's matmuls) plus critical-path work on the cell:

- The stored state is H' = 2h and C' = 2c, so every sigmoid becomes
  0.5*(1+tanh(x/2)) with the (1+t) folded into fused scalar_tensor_tensor
  vector ops and the 0.5 folded EXACTLY into the fp16 weights that consume h
  (W_hh0, W_ih1, W_hh1, W_out).  All activations are Tanh (no ACT table
  churn), same vector-op count as the plain cell:
      u = (tf + 1) * C';  v = (ti + 1) * tg
      C'new = 0.5*u + v;  tc = tanh(0.5*C'new);  H'new = (to + 1) * tc
- The whole post-gate chain runs at [128,128] half-tile granularity and the
  collective staging DMA is split per half, so the gather launches ~1us
  earlier (the chain of cross-engine hops costs ~0.5-1us each).

PSUM: one start=True per 2KB bank per step (start clears has_written for
the whole bank); gate regions pack 4-per-bank.
"""

import numpy as np

import concourse.bass as bass
import concourse.mybir as mybir
import concourse.tile as tile
from concourse.bass_utils import run_bass_kernel_spmd

B, T, D, H, O = 256, 256, 128, 1024, 128
NC = 8
GW = 4                # group width (cores per batch-half group)
HC = H // GW          # 256 h rows per core
B2 = B // 2           # 128 batch cols per group
KH = H // 128         # 8 k-chunks over H
NR = 8                # M-tiles per layer per core: (gate, sub-tile t)
FP = mybir.dt.float16
F32 = mybir.dt.float32
AFT = mybir.ActivationFunctionType
ALU = mybir.AluOpType
RG = [[0, 1, 2, 3], [4, 5, 6, 7]]


def _act_block(nc, apool, gp, bb, c_sb, tag):
    """All-tanh LSTM cell tail for my 256 h rows x 128 batch, processed per
    128-col half-tile t so the chain pipelines.

    gp: 2 psum tiles [128, 512]; region r = gamma*2+t (tile r//4, col region
    r%4) holds gate gamma half t pre-activations WITHOUT bias.  bb: [128,NR]
    fp32; cols for i/f/o gates hold bias/2, g-gate cols hold full bias.
    c_sb: C' = 2c [128, 256] fp32 (col t*128+b), updated in place.
    Returns H' = 2h [128, 256] fp16."""
    t_ = [apool.tile([128, 256], F32, tag=tag + n, name=tag + n)
          for n in "ifgo"]
    it, ft, gt, ot = t_
    u = apool.tile([128, 256], F32, tag=tag + "u")
    v = apool.tile([128, 256], F32, tag=tag + "v")
    tc_ = apool.tile([128, 256], F32, tag=tag + "tc")
    hnew = apool.tile([128, 256], FP, tag=tag + "h")
    sls = [slice(0, 128), slice(128, 256)]

    def reg(gamma, t):
        r = gamma * 2 + t
        return gp[r // 4][:, (r % 4) * 128:(r % 4 + 1) * 128], bb[:, r:r + 1]

    # tf = tanh((gf+bf)/2); ti, to likewise; tg = tanh(gg+bg).  All gate
    # acts first (their psum regions complete in this order), then the
    # fused vector chain per half, then tanh(C') per half, then H' per
    # half — so the two halves pipeline across scalar/vector queues.
    for gamma, dst in ((0, it), (1, ft), (2, gt), (3, ot)):
        for t in (0, 1):
            pre, bias = reg(gamma, t)
            nc.scalar.activation(dst[:, sls[t]], pre, AFT.Tanh, bias=bias,
                                 scale=1.0 if gamma == 2 else 0.5)
    for t in (0, 1):
        sl = sls[t]
        nc.vector.scalar_tensor_tensor(
            u[:, sl], ft[:, sl], 1.0, c_sb[:, sl], ALU.add, ALU.mult)
        nc.vector.scalar_tensor_tensor(
            v[:, sl], it[:, sl], 1.0, gt[:, sl], ALU.add, ALU.mult)
        nc.vector.scalar_tensor_tensor(
            c_sb[:, sl], u[:, sl], 0.5, v[:, sl], ALU.mult, ALU.add)
    for t in (0, 1):
        nc.scalar.activation(tc_[:, sls[t]], c_sb[:, sls[t]], AFT.Tanh,
                             scale=0.5)
    for t in (0, 1):
        nc.vector.scalar_tensor_tensor(
            hnew[:, sls[t]], ot[:, sls[t]], 1.0, tc_[:, sls[t]],
            ALU.add, ALU.mult)
    return hnew


def build(t_steps, debug=False, no_cc=False):
    nc = bass.Bass(num_devices=NC)

    # all tensors are per-core pre-sliced SBUF images (contiguous DMAs)
    xg = nc.dram_tensor("xg", [t_steps, D, B2], FP, kind="ExternalInput")
    w0i = nc.dram_tensor("w0i", [NR, 128, 128], FP, kind="ExternalInput")
    w0h = nc.dram_tensor("w0h", [KH * NR, 128, 128], FP, kind="ExternalInput")
    w1i = nc.dram_tensor("w1i", [KH * NR, 128, 128], FP, kind="ExternalInput")
    w1h = nc.dram_tensor("w1h", [KH * NR, 128, 128], FP, kind="ExternalInput")
    wo = nc.dram_tensor("wo", [KH, 128, O], FP, kind="ExternalInput")
    b0 = nc.dram_tensor("b0", [128, NR], F32, kind="ExternalInput")
    b1 = nc.dram_tensor("b1", [128, NR], F32, kind="ExternalInput")
    bo = nc.dram_tensor("bo", [1, O], FP, kind="ExternalInput")
    ones = nc.dram_tensor("ones", [1, B2], FP, kind="ExternalInput")
    h0i = nc.dram_tensor("h0i", [128, KH * 128], FP, kind="ExternalInput")
    c0i = nc.dram_tensor("c0i", [128, 256], F32, kind="ExternalInput")
    outT = nc.dram_tensor("outT", [O, B2], F32, kind="ExternalOutput")
    if debug:
        dbg_h0 = nc.dram_tensor("dbg_h0", [128, KH * 128], FP, kind="ExternalOutput")
        dbg_h1 = nc.dram_tensor("dbg_h1", [128, KH * 128], FP, kind="ExternalOutput")

    with tile.TileContext(nc) as tc:
        with (
            tc.tile_pool(name="wpool", bufs=1) as wpool,
            tc.tile_pool(name="spool", bufs=1) as spool,
            tc.tile_pool(name="xpool", bufs=6) as xpool,
            tc.tile_pool(name="apool", bufs=3) as apool,
            tc.tile_pool(name="ppool", bufs=1, space="PSUM") as ppool,
            tc.tile_pool(name="dpool", bufs=4, space="DRAM") as dpool,
        ):
            w0i_sb = wpool.tile([128, NR * 128], FP, tag="w0i")
            w0h_sb = wpool.tile([128, KH * NR * 128], FP, tag="w0h")
            w1i_sb = wpool.tile([128, KH * NR * 128], FP, tag="w1i")
            w1h_sb = wpool.tile([128, KH * NR * 128], FP, tag="w1h")
            wo_sb = wpool.tile([128, KH * O], FP, tag="wo")
            b0_sb = wpool.tile([128, NR], F32, tag="b0")
            b1_sb = wpool.tile([128, NR], F32, tag="b1")
            bo_sb = wpool.tile([1, O], FP, tag="bo")
            ones_sb = wpool.tile([1, B2], FP, tag="ones")
            nc.sync.dma_start(w0i_sb[:].rearrange("p (r m) -> p r m", r=NR), w0i[:].rearrange("r p m -> p r m"))
            nc.sync.dma_start(w0h_sb[:].rearrange("p (q m) -> p q m", q=KH * NR), w0h[:].rearrange("q p m -> p q m"))
            nc.sync.dma_start(w1i_sb[:].rearrange("p (q m) -> p q m", q=KH * NR), w1i[:].rearrange("q p m -> p q m"))
            nc.sync.dma_start(w1h_sb[:].rearrange("p (q m) -> p q m", q=KH * NR), w1h[:].rearrange("q p m -> p q m"))
            nc.sync.dma_start(wo_sb[:].rearrange("p (k m) -> p k m", k=KH), wo[:].rearrange("k p m -> p k m"))
            nc.sync.dma_start(b0_sb[:], b0[:])
            nc.sync.dma_start(b1_sb[:], b1[:])
            nc.sync.dma_start(bo_sb[:], bo[:])
            nc.sync.dma_start(ones_sb[:], ones[:])

            # h state double buffers [128, KH*128] (col (w*2+t)*128+b):
            # tick s reads buf s%2, gathers of tick s land in buf (s+1)%2
            h0_sb = [spool.tile([128, KH * 128], FP, tag=f"h0T{i}",
                                name=f"h0T{i}") for i in (0, 1)]
            h1_sb = [spool.tile([128, KH * 128], FP, tag=f"h1T{i}",
                                name=f"h1T{i}") for i in (0, 1)]
            c0_sb = spool.tile([128, 256], F32, tag="c0")
            c1_sb = spool.tile([128, 256], F32, tag="c1")
            nc.sync.dma_start(h0_sb[0][:], h0i[:])
            nc.sync.dma_start(h1_sb[1][:], h0i[:])
            nc.sync.dma_start(c0_sb[:], c0i[:])
            nc.sync.dma_start(c1_sb[:], c0i[:])

            def gates(gp, wi_sb, wh_sb, hin, hrec, n_in_k):
                """Accumulate gate pre-activations for my NR M-tiles.
                hin-dependent matmuls first, hrec-dependent last.

                start=True clears has_written bits for the ENTIRE 2KB PSUM
                bank, so issue exactly one start per bank (first matmul into
                it) and one stop (its last matmul)."""
                for r in range(NR):
                    for k in range(n_in_k):
                        nc.tensor.matmul(
                            gp[r // 4][:, (r % 4) * 128:(r % 4 + 1) * 128],
                            wi_sb[:, (k * NR + r) * 128:(k * NR + r + 1) * 128],
                            hin[:, k * 128:(k + 1) * 128],
                            start=(k == 0 and r % 4 == 0), stop=False)
                for r in range(NR):
                    for k in range(KH):
                        nc.tensor.matmul(
                            gp[r // 4][:, (r % 4) * 128:(r % 4 + 1) * 128],
                            wh_sb[:, (k * NR + r) * 128:(k * NR + r + 1) * 128],
                            hrec[:, k * 128:(k + 1) * 128],
                            start=False,
                            stop=(k == KH - 1 and r % 4 == 3))

            def gather(hnew, h_dst, cci_tag, cco_tag):
                cc_in = dpool.tile([128, 256], FP, tag=cci_tag, name=cci_tag)
                cc_out = dpool.tile([GW, 128, 256], FP, tag=cco_tag, name=cco_tag)
                # stage per half-tile so the DMA starts as soon as that half
                # of the chain finishes
                # stage via the scalar hwdge queue (fast dispatch) instead
                # of gpsimd swdge; the trigger below still rides gpsimd
                nc.scalar.dma_start(cc_in[:, :128], hnew[:, :128])
                nc.scalar.dma_start(cc_in[:, 128:], hnew[:, 128:])
                if not no_cc:
                    nc.gpsimd.collective_compute(
                        "AllGather", mybir.AluOpType.bypass, replica_groups=RG,
                        ins=[cc_in.opt()], outs=[cc_out.opt()])
                nc.sync.dma_start(
                    h_dst[:].rearrange("p (w x) -> p w x", w=GW),
                    cc_out[:].rearrange("w p x -> p w x"))

            for s in range(t_steps + 1):
                p = s % 2
                q = 1 - p
                if s < t_steps:
                    xt = xpool.tile([D, B2], FP, tag="xt")
                    nc.scalar.dma_start(xt[:], xg[s])
                    gp0 = [ppool.tile([128, 512], F32, tag=f"p0{i}",
                                      name=f"p0{i}_{s}") for i in (0, 1)]
                    gates(gp0, w0i_sb, w0h_sb, xt, h0_sb[p], 1)
                    h0new = _act_block(nc, apool, gp0, b0_sb, c0_sb, "l0")
                    gather(h0new, h0_sb[q], "cc0i", "cc0o")
                if s >= 1:
                    gp1 = [ppool.tile([128, 512], F32, tag=f"p1{i}",
                                      name=f"p1{i}_{s}") for i in (0, 1)]
                    gates(gp1, w1i_sb, w1h_sb, h0_sb[p], h1_sb[p], KH)
                    h1new = _act_block(nc, apool, gp1, b1_sb, c1_sb, "l1")
                    gather(h1new, h1_sb[q], "cc1i", "cc1o")

            # output projection for my half: out^T[O, B2] = W_out' @ H1' + b_out
            pfin = (t_steps + 1) % 2
            po = ppool.tile([O, B2], F32, tag="po")
            nc.tensor.matmul(po[:], bo_sb[:], ones_sb[:], start=True, stop=False)
            for k in range(KH):
                nc.tensor.matmul(
                    po[:], wo_sb[:, k * O:(k + 1) * O],
                    h1_sb[pfin][:, k * 128:(k + 1) * 128],
                    start=False, stop=(k == KH - 1))
            out_sb = apool.tile([O, B2], F32, tag="out")
            nc.scalar.copy(out_sb[:], po[:])
            nc.sync.dma_start(outT[:], out_sb[:])
            if debug:
                nc.sync.dma_start(dbg_h0[:], h0_sb[t_steps % 2][:])
                nc.sync.dma_start(dbg_h1[:], h1_sb[pfin][:])

    _split_excess_waits(nc)
    return nc


def _split_excess_waits(nc):
    """This walrus build embeds at most ONE sync wait per instruction (any
    type).  Move excess waits onto same-engine drains inserted immediately
    before the instruction, one wait per drain — engine queues execute in
    order, so semantics are unchanged."""
    for bb in nc.main_func.blocks:
        insts = list(bb.instructions)
        inserts = {}
        extras = []
        for pos, ins in enumerate(insts):
            si = ins.sync_info
            if si is None or not si.on_wait or len(si.on_wait) <= 1:
                continue
            waits = list(si.on_wait)
            keep, excess = waits[-1:], waits[:-1]
            carriers = []
            for w in excess:
                d = nc.engines[ins.engine].drain(fusable=False).ins
                d.sync_info = mybir.SyncInfo(on_wait=[w], on_update=[])
                carriers.append(d)
                extras.append(d)
            inserts[pos] = carriers
            si.on_wait = keep
            ins.sync_info = si
        if not inserts:
            continue
        extra_set = set(id(e) for e in extras)
        for blk in nc.main_func.blocks:
            blk.instructions = [i for i in blk.instructions
                                if id(i) not in extra_set]
        out = []
        for pos, ins in enumerate(insts):
            out.extend(inserts.get(pos, ()))
            out.append(ins)
        bb.instructions = out


def make_in_maps(x, h0, c0, W_ih0, W_hh0, b_ih0, b_hh0,
                 W_ih1, W_hh1, b_ih1, b_hh1, W_out, b_out, t_steps):
    """Per-core SBUF images.  Core j: group g=j//4 (batch half), member
    m=j%4 (owns h rows [m*256, (m+1)*256)).  State is scaled: H'=2h, C'=2c;
    weights that consume h carry a 0.5 factor (exact in fp16)."""
    wo_host = np.ascontiguousarray(
        0.5 * W_out.T).astype(np.float16).reshape(KH, 128, O)
    bo_host = b_out.astype(np.float16).reshape(1, O)

    def wimg(W, m, kin, scale):
        out = np.empty((kin * NR, 128, 128), np.float16)
        for k in range(kin):
            for r in range(NR):
                gamma, t = r // 2, r % 2
                rows = slice(gamma * H + m * HC + t * 128,
                             gamma * H + m * HC + (t + 1) * 128)
                out[k * NR + r] = (
                    scale * W[rows, k * 128:(k + 1) * 128].T).astype(np.float16)
        return out

    def bimg(bvec, m):
        # [128, NR] fp32; i/f/o-gate cols pre-halved (sigmoid-via-tanh),
        # g-gate cols full
        out = np.empty((128, NR), np.float32)
        for r in range(NR):
            gamma, t = r // 2, r % 2
            bb = bvec[gamma * H + m * HC + t * 128:
                      gamma * H + m * HC + (t + 1) * 128]
            out[:, r] = bb if gamma == 2 else 0.5 * bb
        return out

    def himg(hT_half):
        # H' = 2*h initial, [128, KH*128] col k*128+b <- 2*hT[k*128+p, b]
        return np.ascontiguousarray(
            (2.0 * hT_half).reshape(KH, 128, B2).transpose(1, 0, 2)
            .reshape(128, KH * B2)).astype(np.float16)

    def cimg(cT_half, m):
        # C' = 2*c initial, my 256 rows, [128, 256] col t*128+b
        chunk = 2.0 * cT_half[m * HC:(m + 1) * HC]
        return np.ascontiguousarray(
            chunk.reshape(2, 128, B2).transpose(1, 0, 2).reshape(128, 256)
        ).astype(np.float32)

    h0T = h0.T.astype(np.float32)
    c0T = c0.T.astype(np.float32)
    b0v = (b_ih0 + b_hh0).astype(np.float32)
    b1v = (b_ih1 + b_hh1).astype(np.float32)
    xT = np.transpose(x[:, :t_steps, :], (1, 2, 0)).astype(np.float16)

    in_maps = []
    for j in range(NC):
        g, m = j // GW, j % GW
        sl = slice(g * B2, (g + 1) * B2)
        in_maps.append({
            "xg": np.ascontiguousarray(xT[:, :, sl]),
            "w0i": wimg(W_ih0, m, 1, 1.0),
            "w0h": wimg(W_hh0, m, KH, 0.5),
            "w1i": wimg(W_ih1, m, KH, 0.5),
            "w1h": wimg(W_hh1, m, KH, 0.5),
            "wo": wo_host, "bo": bo_host,
            "b0": bimg(b0v, m), "b1": bimg(b1v, m),
            "ones": np.ones((1, B2), np.float16),
            "h0i": himg(h0T[:, sl]),
            "c0i": cimg(c0T[:, sl], m),
        })
    return in_maps


def run(t_steps, in_maps, trace=False):
    nc = build(t_steps)
    res = run_bass_kernel_spmd(nc, in_maps, list(range(NC)), trace=trace)
    return res


def assemble(res):
    out = np.concatenate(
        [res.results[0]["outT"].T, res.results[GW]["outT"].T], axis=0)
    return np.ascontiguousarray(out).astype(np.float32)


def kernel(**inputs):
    args = {k: np.asarray(v) for k, v in inputs.items()}
    in_maps = make_in_maps(
        args["x"], args["h0"], args["c0"],
        args["W_ih0"], args["W_hh0"], args["b_ih0"], args["b_hh0"],
        args["W_ih1"], args["W_hh1"], args["b_ih1"], args["b_hh1"],
        args["W_out"], args["b_out"], T)
    res = run(T, in_maps)
    return assemble(res)
